# revision 25
# baseline (speedup 1.0000x reference)
"""Trainium2 Bass kernel for LocalDualDirectedMessagePassingLayer.

Strategy (8 cores, dest-sharded, window-compressed aggregation):
  - 64 dest blocks of 128 dests; blocks sorted by edge count desc and grouped
    into 8 "slots" of 8 similar-sized blocks, one block per core per slot.
    Slot k is padded to caps[k] 128-edge subtiles (max over its 8 blocks),
    so per-core padding is ~the within-slot size spread (small).
  - dest_seg is sorted, so a 128-edge subtile touches only a narrow window of
    dests.  The scaled one-hot aggregation matrix is stored as a [128, W=32]
    window per subtile (base chosen at pack time, identical across cores =>
    SPMD-safe), cutting its DRAM traffic 4x vs a full [128,128] one-hot.
    The first subtile of each block uses a wide [128,160] slice with start=True
    to zero+init the PSUM accumulator; later subtiles accumulate (start=False)
    into [base:base+W] sub-windows.
  - Device per 512-edge super-tile: read MLP out [j,512] via 2 K-tile matmuls
    + ACT relu(+b_read); per 128-edge sub-tile: msg MLP out [e,128], relu
    alternating vector/scalar engines, then agg matmul accumulates
    msg_mean^T into the block's PSUM window.
  - All bulk DMA rides the two hardware DGE queues (sync + scalar engines),
    alternating tensor assignment per chunk for byte balance, with 8KB
    per-partition packets (8-super-tile chunks).  Consts + output writes go
    to the gpsimd software queue.
  - Per block: dst-side MLP chain (agg/upd/write) -> tanh -> writeT [128,1024],
    interleaved with the next block's subtiles.
  - Host: transpose writeT, scatter rows into a copy of node_memory.
All matmul operands bf16, PSUM accumulation fp32.
"""

import sys

sys.path.insert(0, "/opt/trn_rl_repo")

import math

import ml_dtypes
import numpy as np

import concourse.bass as bass
import concourse.mybir as mybir
import concourse.tile as tile
from concourse import bacc
from concourse.bass_utils import run_bass_kernel_spmd

BF16 = ml_dtypes.bfloat16
FP8 = ml_dtypes.float8_e4m3
N_CORES = 8
SUP = 512
P = 128
N_DEST = 8192
D_MEM = 128
WIDE = 160          # PSUM agg region columns (128 dests + window slack)
CH = 8              # super-tiles per DMA chunk (8KB per-partition packets)

_PROG_CACHE: dict[tuple, object] = {}


def _build_program(caps, bases, W):
    """SPMD Bass program. caps[k] = subtiles in slot k; bases[t] = static
    window base for global subtile t (ignored for the first subtile of each
    block, which uses the wide slice)."""
    blk_of, qin_of = [], []
    for k, ck in enumerate(caps):
        blk_of += [k] * ck
        qin_of += list(range(ck))
    T = len(blk_of)
    nsup = (T + 3) // 4
    T4 = nsup * 4
    e_cap = nsup * SUP
    nch = (nsup + CH - 1) // CH

    nc = bacc.Bacc("TRN2", target_bir_lowering=False, debug=False,
                   num_devices=N_CORES)
    f32 = mybir.dt.float32
    bf16 = mybir.dt.bfloat16
    fp8 = mybir.dt.float8e4
    AF = mybir.ActivationFunctionType

    # chunk-major layouts: each chunk's slab is contiguous in DRAM.
    # s01 packs both read-MLP k-tiles planar per chunk for DoubleRow matmul.
    s01_d = nc.dram_tensor("s01", [nch, P, 2, CH * SUP], fp8,
                           kind="ExternalInput")
    efts = nc.dram_tensor("efts", [nch, P, CH * SUP], fp8,
                          kind="ExternalInput")
    S_d = nc.dram_tensor("S_d", [nch, P, CH * 4 * W], fp8,
                         kind="ExternalInput")
    Sw_d = nc.dram_tensor("Sw_d", [P, 8 * WIDE], fp8, kind="ExternalInput")
    dstT = nc.dram_tensor("dstT", [2, P, 1024], bf16, kind="ExternalInput")
    wr = nc.dram_tensor("wr", [2, P, P], bf16, kind="ExternalInput")
    wr8 = nc.dram_tensor("wr8", [P, 2, P], fp8, kind="ExternalInput")
    wm01 = nc.dram_tensor("wm01", [P, 2, P], fp8, kind="ExternalInput")
    wa = nc.dram_tensor("wa", [2, P, P], bf16, kind="ExternalInput")
    wu = nc.dram_tensor("wu", [2, P, P], bf16, kind="ExternalInput")
    ww = nc.dram_tensor("ww", [P, P], bf16, kind="ExternalInput")
    br = nc.dram_tensor("br", [P, 1], f32, kind="ExternalInput")
    ba = nc.dram_tensor("ba", [P, 1], f32, kind="ExternalInput")
    bu = nc.dram_tensor("bu", [P, 1], f32, kind="ExternalInput")
    bw = nc.dram_tensor("bw", [P, 1], f32, kind="ExternalInput")
    out_d = nc.dram_tensor("writeT", [P, 1024], f32, kind="ExternalOutput")

    with tile.TileContext(nc) as tc:
        with (
            tc.tile_pool(name="const", bufs=1) as cp,
            tc.tile_pool(name="io", bufs=5) as iop,
            tc.tile_pool(name="msp", bufs=3) as msp,
            tc.tile_pool(name="dmid", bufs=2) as dmid,
            tc.tile_pool(name="rdps", bufs=2, space="PSUM") as rdps,
            tc.tile_pool(name="mgps", bufs=3, space="PSUM") as mgps,
            tc.tile_pool(name="aggps", bufs=2, space="PSUM") as aggps,
            tc.tile_pool(name="dstps", bufs=1, space="PSUM") as dstps,
        ):
            def cload(eng, ap, shape, dtype, tag):
                t = cp.tile(shape, dtype, tag=tag, name=tag)
                eng.dma_start(out=t[:], in_=ap)
                return t

            # scalar engine must issue NO const DMAs: its HWDGE ring waits
            # would block the ACT relus queued behind them (in-order engine).
            br_t = cload(nc.sync, br[:, :], [P, 1], f32, "br")
            wr8_t = cload(nc.sync, wr8[:, :, :], [P, 2, P], fp8, "wr8")
            swide = cload(nc.gpsimd, Sw_d[:, :], [P, 8 * WIDE], fp8, "swide")
            wm01_t = cload(nc.gpsimd, wm01[:, :, :], [P, 2, P], fp8, "wm01")
            dstT0 = cp.tile([P, 1024], bf16, tag="dstT0", name="dstT0")
            dstT1 = cp.tile([P, 1024], bf16, tag="dstT1", name="dstT1")
            for r in (0, 64):
                nc.sync.dma_start(out=dstT0[r:r + 64, :], in_=dstT[0, r:r + 64, :])
                nc.sync.dma_start(out=dstT1[r:r + 64, :], in_=dstT[1, r:r + 64, :])

            def dst_stage(b, agg_ps, stage, hold):
                dc = slice(b * P, (b + 1) * P)
                if stage == 0:
                    mmean = dmid.tile([P, P], bf16, tag="mmean")
                    nc.vector.tensor_copy(mmean[:], agg_ps[:, :P])
                    drp = dstps.tile([P, P], f32, tag="dst")
                    nc.tensor.matmul(drp[:], lhsT=wr0[:], rhs=dstT0[:, dc],
                                     start=True, stop=False)
                    nc.tensor.matmul(drp[:], lhsT=wr1[:], rhs=dstT1[:, dc],
                                     start=False, stop=True)
                    dstr = dmid.tile([P, P], bf16, tag="dstr")
                    nc.scalar.activation(dstr[:], drp[:], AF.Relu, bias=br_t[:, :1])
                    hold.update(mmean=mmean, dstr=dstr)
                elif stage == 1:
                    agp = dstps.tile([P, P], f32, tag="dst")
                    nc.tensor.matmul(agp[:], lhsT=wa0[:], rhs=hold["dstr"][:],
                                     start=True, stop=False)
                    nc.tensor.matmul(agp[:], lhsT=wa1[:], rhs=hold["mmean"][:],
                                     start=False, stop=True)
                    aggT = dmid.tile([P, P], bf16, tag="aggT")
                    nc.scalar.activation(aggT[:], agp[:], AF.Relu, bias=ba_t[:, :1])
                    hold.update(aggT=aggT)
                elif stage == 2:
                    upp = dstps.tile([P, P], f32, tag="dst")
                    nc.tensor.matmul(upp[:], lhsT=wu0[:], rhs=hold["aggT"][:],
                                     start=True, stop=False)
                    nc.tensor.matmul(upp[:], lhsT=wu1[:], rhs=hold["dstr"][:],
                                     start=False, stop=True)
                    updT = dmid.tile([P, P], bf16, tag="updT")
                    nc.scalar.activation(updT[:], upp[:], AF.Relu, bias=bu_t[:, :1])
                    hold.update(updT=updT)
                else:
                    wrp = dstps.tile([P, P], f32, tag="dst")
                    nc.tensor.matmul(wrp[:], lhsT=ww_t[:], rhs=hold["updT"][:],
                                     start=True, stop=True)
                    wout = dmid.tile([P, P], f32, tag="wout")
                    nc.scalar.activation(wout[:], wrp[:], AF.Tanh, bias=bw_t[:, :1])
                    nc.sync.dma_start(out=out_d[:, dc], in_=wout[:])

            def issue_chunk(ci):
                # DRAM slabs are chunk-contiguous.  s01/ef spread across the
                # SWDGE path (gpsimd) and the two HWDGE queues for balance.
                # mi plane 0 is filled by the read-MLP relu on ACT; plane 1
                # is the (zero-padded) edge-feature k-tile, DMA'd here.
                if ci >= nch:
                    return None
                s_lo = ci * CH
                s_hi = min(nsup, s_lo + CH)
                w_cols = (s_hi - s_lo) * SUP
                s01c = iop.tile([P, 2, CH * SUP], fp8, tag="s01")
                mi = iop.tile([P, 2, CH * SUP], fp8, tag="mi")
                Sc = iop.tile([P, CH * 4 * W], fp8, tag="Sc")
                if ci in (1, 5, 7):
                    for r in range(0, P, 64):
                        nc.sync.dma_start(out=s01c[r:r + 64, :, :w_cols],
                                          in_=s01_d[ci, r:r + 64, :, :w_cols])
                elif ci < 2:
                    # ramp: split so early super-tiles can start sooner
                    qn = CH * SUP // 4
                    for j in range(4):
                        c0, c1 = j * qn, min((j + 1) * qn, w_cols)
                        if c0 < c1:
                            nc.gpsimd.dma_start(out=s01c[:, :, c0:c1],
                                                in_=s01_d[ci, :, :, c0:c1])
                else:
                    nc.gpsimd.dma_start(out=s01c[:, :, :w_cols],
                                        in_=s01_d[ci, :, :, :w_cols])
                if ci % 2 == 0:
                    for (r0, r1) in ((0, 64), (64, P)):
                        nc.scalar.dma_start(out=mi[r0:r1, 1, :w_cols],
                                            in_=efts[ci, r0:r1, :w_cols])
                else:
                    nc.gpsimd.dma_start(out=mi[:, 1, :w_cols],
                                        in_=efts[ci, :, :w_cols])
                sw = (s_hi - s_lo) * 4 * W
                nc.sync.dma_start(out=Sc[:, :sw], in_=S_d[ci, :, :sw])
                return (s_lo, s_hi, s01c, mi, Sc)

            PF = 4                  # chunks issued ahead of compute
            chunks = [issue_chunk(0)]
            wr0 = cload(nc.gpsimd, wr[0, :, :], [P, P], bf16, "wr0")
            wr1 = cload(nc.gpsimd, wr[1, :, :], [P, P], bf16, "wr1")
            wa0 = cload(nc.gpsimd, wa[0, :, :], [P, P], bf16, "wa0")
            wa1 = cload(nc.gpsimd, wa[1, :, :], [P, P], bf16, "wa1")
            wu0 = cload(nc.gpsimd, wu[0, :, :], [P, P], bf16, "wu0")
            wu1 = cload(nc.gpsimd, wu[1, :, :], [P, P], bf16, "wu1")
            ww_t = cload(nc.gpsimd, ww[:, :], [P, P], bf16, "ww")
            ba_t = cload(nc.gpsimd, ba[:, :], [P, 1], f32, "ba")
            bu_t = cload(nc.gpsimd, bu[:, :], [P, 1], f32, "bu")
            bw_t = cload(nc.gpsimd, bw[:, :], [P, 1], f32, "bw")
            chunks += [issue_chunk(ci) for ci in range(1, PF)]

            pending = None          # [slot, agg_tile, next_stage]
            hold = {}
            cur_agg = None
            stage_q = []            # software pipeline: read -> msg -> agg
            st_ctx = {}

            def emit_msg(ent):
                s, nq, mi, Sc, s_lo, _ = ent
                sl_sup = (s - s_lo) * SUP
                mg = mgps.tile([P, SUP], f32, tag="mg")
                for q in range(nq):
                    qs = slice(q * P, (q + 1) * P)
                    nc.tensor.matmul(mg[:, qs],
                                     lhsT=mi[:, :, sl_sup + q * P:
                                             sl_sup + (q + 1) * P],
                                     rhs=wm01_t[:], start=True, stop=True,
                                     skip_group_check=True,
                                     perf_mode=mybir.MatmulPerfMode.DoubleRow)
                msgs = msp.tile([P, SUP], fp8, tag="msgs")
                nc.vector.tensor_scalar_max(msgs[:, :nq * P],
                                            mg[:, :nq * P], 0.0)
                st_ctx[s] = msgs

            def emit_agg(ent):
                nonlocal pending, hold, cur_agg
                s, nq, mi, Sc, s_lo, _ = ent
                msgs = st_ctx.pop(s)
                for q in range(nq):
                    t = s * 4 + q
                    k, qin, base = blk_of[t], qin_of[t], bases[t]
                    qs = slice(q * P, (q + 1) * P)
                    last = qin == caps[k] - 1
                    if qin == 0:
                        cur_agg = aggps.tile([P, WIDE], f32, tag="agg")
                        nc.tensor.matmul(cur_agg[:, :],
                                         lhsT=msgs[:, qs],
                                         rhs=swide[:, k * WIDE:(k + 1) * WIDE],
                                         start=True, stop=last,
                                         skip_group_check=True)
                    else:
                        nc.tensor.matmul(cur_agg[:, base:base + W],
                                         lhsT=msgs[:, qs],
                                         rhs=Sc[:, (t - s_lo * 4) * W:
                                                 (t - s_lo * 4 + 1) * W],
                                         start=False, stop=last,
                                         skip_group_check=True)
                    if pending is not None and qin in (2, 6, 10, 14):
                        dst_stage(pending[0], pending[1], pending[2], hold)
                        pending[2] += 1
                        if pending[2] == 4:
                            pending = None
                    if last:
                        if pending is not None:   # tiny-cap fallback: flush
                            while pending[2] < 4:
                                dst_stage(pending[0], pending[1],
                                          pending[2], hold)
                                pending[2] += 1
                        pending = [k, cur_agg, 0]
                        hold = {}

            for ci in range(nch):
                s_lo, s_hi, s01c, mi, Sc = chunks[ci]
                nxt = issue_chunk(ci + PF)
                if nxt is not None:
                    chunks.append(nxt)

                for s in range(s_lo, s_hi):
                    sl = s - s_lo
                    col = slice(sl * SUP, (sl + 1) * SUP)
                    nq = min(T - s * 4, 4)      # valid subtiles this super-tile
                    rd = rdps.tile([P, SUP], f32, tag="rd")
                    nc.tensor.matmul(rd[:], lhsT=wr8_t[:], rhs=s01c[:, :, col],
                                     start=True, stop=True,
                                     perf_mode=mybir.MatmulPerfMode.DoubleRow)
                    nc.scalar.activation(mi[:, 0, col], rd[:], AF.Relu,
                                         bias=br_t[:, :1])
                    stage_q.append((s, nq, mi, Sc, s_lo, None))
                    if len(stage_q) >= 2:
                        emit_msg(stage_q[-2])
                    if len(stage_q) >= 3:
                        emit_agg(stage_q.pop(0))
            for ent in stage_q:
                if ent[0] not in st_ctx:
                    emit_msg(ent)
            while stage_q:
                emit_agg(stage_q.pop(0))
            while pending[2] < 4:
                dst_stage(pending[0], pending[1], pending[2], hold)
                pending[2] += 1

    nc.finalize()
    return nc


def _pack(dest_seg, bounds, assign, caps, W):
    """Lockstep-pack each slot's 8 blocks into caps[k] subtiles with shared
    static window bases.  Returns (bases, takes) or None if W too narrow.
    takes[t] = list of (core, edge_lo, count, base)."""
    bases, takes = [], []
    for k in range(8):
        lo = np.array([bounds[assign[c, k]] for c in range(8)], np.int64)
        hi = np.array([bounds[assign[c, k] + 1] for c in range(8)], np.int64)
        blk0 = np.array([assign[c, k] * P for c in range(8)], np.int64)
        ptr = lo.copy()
        for q in range(caps[k]):
            pend = ptr < hi
            if pend.any():
                base = int(min(dest_seg[ptr[c]] - blk0[c]
                               for c in range(8) if pend[c]))
            else:
                base = 0
            if q == 0 or W >= P:
                base, weff = 0, P
            else:
                # window must stay inside the [0, WIDE) PSUM region
                base = min(base, WIDE - W)
                weff = W
            rec = []
            for c in range(8):
                if not pend[c]:
                    rec.append((c, int(ptr[c]), 0, base))
                    continue
                dl = dest_seg[ptr[c]:hi[c]] - blk0[c]
                nfit = int(np.searchsorted(dl, base + weff))
                tc = min(P, nfit)
                rec.append((c, int(ptr[c]), tc, base))
                ptr[c] += tc
            bases.append(base)
            takes.append(rec)
        if (ptr != hi).any():
            return None
    return bases, takes


def _prep_inputs(inputs):
    """Host-side shard/pack. Returns (in_maps, key, node_memory, node_ids,
    dest_cols)."""
    node_memory = np.ascontiguousarray(np.asarray(inputs["node_memory"], np.float32))
    node_features = np.asarray(inputs["node_features"], np.float32)
    edge_features = np.asarray(inputs["edge_features"], np.float32)
    time_encoding = np.asarray(inputs["time_encoding"], np.float32)
    node_ids = np.asarray(inputs["node_ids"]).astype(np.int64)
    source_ids = np.asarray(inputs["source_ids"]).astype(np.int64)
    edge_ids = np.asarray(inputs["edge_ids"]).astype(np.int64)
    dest_seg = np.asarray(inputs["dest_seg"]).astype(np.int64)
    W_read = np.asarray(inputs["W_read"], np.float32)
    b_read = np.asarray(inputs["b_read"], np.float32)
    W_msg = np.asarray(inputs["W_msg"], np.float32)
    b_msg = np.asarray(inputs["b_msg"], np.float32)
    W_agg = np.asarray(inputs["W_agg"], np.float32)
    b_agg = np.asarray(inputs["b_agg"], np.float32)
    W_upd = np.asarray(inputs["W_upd"], np.float32)
    b_upd = np.asarray(inputs["b_upd"], np.float32)
    W_write = np.asarray(inputs["W_write"], np.float32)
    b_write = np.asarray(inputs["b_write"], np.float32)

    n_edge = dest_seg.shape[0]

    cnt = np.bincount(dest_seg, minlength=N_DEST)
    inv_cnt = np.zeros(N_DEST, np.float32)
    nz = cnt > 0
    inv_cnt[nz] = 1.0 / cnt[nz]

    # 64 global dest blocks of 128 dests; sort desc by edge count into slots
    bounds = np.searchsorted(dest_seg, np.arange(0, N_DEST + 1, P))
    n_b = np.diff(bounds)
    order = np.argsort(-n_b, kind="stable")
    assign = np.zeros((8, 8), np.int64)     # [core, slot] -> global block
    for k in range(8):
        grp = np.sort(order[8 * k:8 * (k + 1)])
        assign[:, k] = grp
    caps = tuple(int(math.ceil(max(n_b[assign[c, k]] for c in range(8)) / P))
                 for k in range(8))

    packed = None
    for W in (32, 64, 128):
        packed = _pack(dest_seg, bounds, assign, caps, W)
        if packed is not None:
            break
    assert packed is not None
    bases, takes = packed
    T = len(bases)
    nsup = (T + 3) // 4
    T4 = nsup * 4
    e_cap = nsup * SUP

    blk_of, qin_of = [], []
    for k, ck in enumerate(caps):
        blk_of += [k] * ck
        qin_of += list(range(ck))

    # per-core flat edge selection
    esel = np.zeros((N_CORES, e_cap), np.int64)
    valid = np.zeros((N_CORES, e_cap), bool)
    for t, rec in enumerate(takes):
        for (c, elo, tc, base) in rec:
            if tc:
                esel[c, t * P:t * P + tc] = np.arange(elo, elo + tc)
                valid[c, t * P:t * P + tc] = True
    esel_f = esel.reshape(-1)
    valid_f = valid.reshape(-1)

    nodecat = np.concatenate([node_memory, node_features], axis=1)  # [N,256]

    nch = (nsup + CH - 1) // CH
    e_pad = nch * CH * SUP          # chunk-major padded edge capacity

    def chunk_major(arr):
        """[N_CORES, R, e_cap] -> [N_CORES, nch, R, CH*SUP] contiguous."""
        n, r = arr.shape[0], arr.shape[1]
        out = np.zeros((n, r, e_pad), arr.dtype)
        out[:, :, :e_cap] = arr
        return np.ascontiguousarray(
            out.reshape(n, r, nch, CH * SUP).transpose(0, 2, 1, 3))

    src_rows = nodecat[source_ids[esel_f]]
    src_rows[~valid_f] = 0.0
    srcT = np.ascontiguousarray(
        src_rows.reshape(N_CORES, e_cap, 256).transpose(0, 2, 1)
    ).astype(FP8).reshape(N_CORES, 2, P, e_cap)
    # planar k-pair layout for DoubleRow: [N, nch, P, 2, CH*SUP]
    s01_pad = np.zeros((N_CORES, 2, P, e_pad), FP8)
    s01_pad[:, :, :, :e_cap] = srcT
    s01 = np.ascontiguousarray(
        s01_pad.reshape(N_CORES, 2, P, nch, CH * SUP)
        .transpose(0, 3, 2, 1, 4))

    ef_rows = edge_features[edge_ids[esel_f]]
    t_rows = time_encoding[np.minimum(esel_f, n_edge - 1)]
    eft = np.concatenate(
        [ef_rows, t_rows, np.ones((len(esel_f), 1), np.float32),
         np.zeros((len(esel_f), 31), np.float32)], axis=1)
    eft[~valid_f] = 0.0
    eft[valid_f, 96] = 1.0
    efts = np.ascontiguousarray(
        eft.reshape(N_CORES, e_cap, P).transpose(0, 2, 1)).astype(FP8)
    efts = chunk_major(efts)                      # [N, nch, 128, CH*SUP]

    # windowed scaled one-hot S + per-block wide first-subtile slice
    W_used = W          # width that succeeded in the pack loop above
    S_np = np.zeros((N_CORES, P, nch * CH * 4 * W_used), np.float32)
    Sw_np = np.zeros((N_CORES, P, 8 * WIDE), np.float32)
    for t, rec in enumerate(takes):
        k, qin = blk_of[t], qin_of[t]
        for (c, elo, tc, base) in rec:
            if not tc:
                continue
            dl = (dest_seg[elo:elo + tc] - assign[c, k] * P).astype(np.int64)
            sc = inv_cnt[dest_seg[elo:elo + tc]]
            e_i = np.arange(tc)
            if qin == 0:
                Sw_np[c, e_i, k * WIDE + dl] = sc
            else:
                S_np[c, e_i, t * W_used + (dl - base)] = sc
    S_np = np.ascontiguousarray(
        S_np.reshape(N_CORES, P, nch, CH * 4 * W_used).transpose(0, 2, 1, 3)
    ).astype(FP8)                                 # [N, nch, P, CH*4*W]
    Sw_np = Sw_np.astype(FP8)

    # dst-side node rows, in slot order per core
    dest_cols = np.zeros((N_CORES, 1024), np.int64)   # dest index per out col
    for c in range(N_CORES):
        for k in range(8):
            dest_cols[c, k * P:(k + 1) * P] = assign[c, k] * P + np.arange(P)
    drows = nodecat[node_ids[dest_cols.reshape(-1)]]
    dstT = np.ascontiguousarray(
        drows.reshape(N_CORES, 1024, 256).transpose(0, 2, 1)
    ).astype(BF16).reshape(N_CORES, 2, P, 1024)

    wr_h = np.ascontiguousarray(W_read.reshape(2, P, P)).astype(BF16)
    wr8_h = np.ascontiguousarray(
        W_read.reshape(2, P, P).transpose(1, 0, 2)).astype(FP8)  # [P,2,P]
    wm01_h = np.zeros((P, 2, P), np.float32)
    wm01_h[:, 0, :] = W_msg[:P]
    wm01_h[:97, 1, :] = np.concatenate([W_msg[P:], b_msg[None, :]], axis=0)
    wm01_h = wm01_h.astype(FP8)
    wa_h = np.ascontiguousarray(W_agg.reshape(2, P, P)).astype(BF16)
    wu_h = np.ascontiguousarray(W_upd.reshape(2, P, P)).astype(BF16)
    ww_h = np.ascontiguousarray(W_write).astype(BF16)
    br_h = np.ascontiguousarray(b_read[:, None]).astype(np.float32)
    ba_h = np.ascontiguousarray(b_agg[:, None]).astype(np.float32)
    bu_h = np.ascontiguousarray(b_upd[:, None]).astype(np.float32)
    bw_h = np.ascontiguousarray(b_write[:, None]).astype(np.float32)

    in_maps = []
    for c in range(N_CORES):
        in_maps.append({
            "s01": s01[c], "efts": efts[c], "S_d": S_np[c], "Sw_d": Sw_np[c],
            "dstT": dstT[c],
            "wr": wr_h, "wr8": wr8_h, "wm01": wm01_h, "wa": wa_h, "wu": wu_h,
            "ww": ww_h, "br": br_h, "ba": ba_h, "bu": bu_h, "bw": bw_h,
        })
    key = (caps, tuple(bases), W_used)
    return in_maps, key, node_memory, node_ids, dest_cols


def run(inputs, trace=False, **kw):
    in_maps, key, node_memory, node_ids, dest_cols = _prep_inputs(inputs)
    if key not in _PROG_CACHE:
        _PROG_CACHE[key] = _build_program(key[0], key[1], key[2])
    nc = _PROG_CACHE[key]
    res = run_bass_kernel_spmd(nc, in_maps, core_ids=list(range(N_CORES)),
                               trace=trace, **kw)
    out = node_memory.copy()
    for c in range(N_CORES):
        wt = np.asarray(res.results[c]["writeT"], np.float32).T  # [1024,128]
        out[node_ids[dest_cols[c]]] = wt
    return out, res


def kernel(**inputs) -> np.ndarray:
    out, _ = run(inputs, trace=False)
    return out


# revision 30
# speedup vs baseline: 1.0587x; 1.0587x over previous
"""Trainium2 Bass kernel for LocalDualDirectedMessagePassingLayer.

Strategy (8 cores, dest-sharded, window-compressed aggregation):
  - 64 dest blocks of 128 dests; blocks sorted by edge count desc and grouped
    into 8 "slots" of 8 similar-sized blocks, one block per core per slot.
    Slot k is padded to caps[k] 128-edge subtiles (max over its 8 blocks),
    so per-core padding is ~the within-slot size spread (small).
  - dest_seg is sorted, so a 128-edge subtile touches only a narrow window of
    dests.  The scaled one-hot aggregation matrix is stored as a [128, W=32]
    window per subtile (base chosen at pack time, identical across cores =>
    SPMD-safe), cutting its DRAM traffic 4x vs a full [128,128] one-hot.
    The first subtile of each block uses a wide [128,160] slice with start=True
    to zero+init the PSUM accumulator; later subtiles accumulate (start=False)
    into [base:base+W] sub-windows.
  - Device per 512-edge super-tile: read MLP out [j,512] via 2 K-tile matmuls
    + ACT relu(+b_read); per 128-edge sub-tile: msg MLP out [e,128], relu
    alternating vector/scalar engines, then agg matmul accumulates
    msg_mean^T into the block's PSUM window.
  - All bulk DMA rides the two hardware DGE queues (sync + scalar engines),
    alternating tensor assignment per chunk for byte balance, with 8KB
    per-partition packets (8-super-tile chunks).  Consts + output writes go
    to the gpsimd software queue.
  - Per block: dst-side MLP chain (agg/upd/write) -> tanh -> writeT [128,1024],
    interleaved with the next block's subtiles.
  - Host: transpose writeT, scatter rows into a copy of node_memory.
All matmul operands bf16, PSUM accumulation fp32.
"""

import sys

sys.path.insert(0, "/opt/trn_rl_repo")

import math

import ml_dtypes
import numpy as np

import concourse.bass as bass
import concourse.mybir as mybir
import concourse.tile as tile
from concourse import bacc
from concourse.bass_utils import run_bass_kernel_spmd

BF16 = ml_dtypes.bfloat16
FP8 = ml_dtypes.float8_e4m3
N_CORES = 8
SUP = 512
P = 128
N_DEST = 8192
D_MEM = 128
WIDE = 160          # PSUM agg region columns (128 dests + window slack)
CH = 8              # super-tiles per DMA chunk (8KB per-partition packets)

_PROG_CACHE: dict[tuple, object] = {}


def _build_program(caps, bases, W):
    """SPMD Bass program. caps[k] = subtiles in slot k; bases[t] = static
    window base for global subtile t (ignored for the first subtile of each
    block, which uses the wide slice)."""
    blk_of, qin_of = [], []
    for k, ck in enumerate(caps):
        blk_of += [k] * ck
        qin_of += list(range(ck))
    T = len(blk_of)
    nsup = (T + 3) // 4
    T4 = nsup * 4
    e_cap = nsup * SUP
    nch = (nsup + CH - 1) // CH

    nc = bacc.Bacc("TRN2", target_bir_lowering=False, debug=False,
                   num_devices=N_CORES)
    f32 = mybir.dt.float32
    bf16 = mybir.dt.bfloat16
    fp8 = mybir.dt.float8e4
    AF = mybir.ActivationFunctionType

    # chunk-major layouts: each chunk's slab is contiguous in DRAM.
    # s01 packs both read-MLP k-tiles planar per chunk for DoubleRow matmul.
    s01_d = nc.dram_tensor("s01", [nch, P, 2, CH * SUP], fp8,
                           kind="ExternalInput")
    efts = nc.dram_tensor("efts", [nch, P, CH * SUP], fp8,
                          kind="ExternalInput")
    S_d = nc.dram_tensor("S_d", [nch, P, CH * 4 * W], fp8,
                         kind="ExternalInput")
    Sw_d = nc.dram_tensor("Sw_d", [P, 8 * WIDE], fp8, kind="ExternalInput")
    dstT = nc.dram_tensor("dstT", [2, P, 1024], bf16, kind="ExternalInput")
    wr = nc.dram_tensor("wr", [2, P, P], bf16, kind="ExternalInput")
    wr8 = nc.dram_tensor("wr8", [P, 2, P], fp8, kind="ExternalInput")
    wm01 = nc.dram_tensor("wm01", [P, 2, P], fp8, kind="ExternalInput")
    wa = nc.dram_tensor("wa", [2, P, P], bf16, kind="ExternalInput")
    wu = nc.dram_tensor("wu", [2, P, P], bf16, kind="ExternalInput")
    ww = nc.dram_tensor("ww", [P, P], bf16, kind="ExternalInput")
    br = nc.dram_tensor("br", [P, 1], f32, kind="ExternalInput")
    ba = nc.dram_tensor("ba", [P, 1], f32, kind="ExternalInput")
    bu = nc.dram_tensor("bu", [P, 1], f32, kind="ExternalInput")
    bw = nc.dram_tensor("bw", [P, 1], f32, kind="ExternalInput")
    out_d = nc.dram_tensor("writeT", [P, 1024], f32, kind="ExternalOutput")

    with tile.TileContext(nc) as tc:
        with (
            tc.tile_pool(name="const", bufs=1) as cp,
            tc.tile_pool(name="io", bufs=5) as iop,
            tc.tile_pool(name="msp", bufs=3) as msp,
            tc.tile_pool(name="dmid", bufs=2) as dmid,
            tc.tile_pool(name="rdps", bufs=2, space="PSUM") as rdps,
            tc.tile_pool(name="mgps", bufs=3, space="PSUM") as mgps,
            tc.tile_pool(name="aggps", bufs=2, space="PSUM") as aggps,
            tc.tile_pool(name="dstps", bufs=1, space="PSUM") as dstps,
        ):
            def cload(eng, ap, shape, dtype, tag):
                t = cp.tile(shape, dtype, tag=tag, name=tag)
                eng.dma_start(out=t[:], in_=ap)
                return t

            # scalar engine must issue NO const DMAs: its HWDGE ring waits
            # would block the ACT relus queued behind them (in-order engine).
            br_t = cload(nc.sync, br[:, :], [P, 1], f32, "br")
            wr8_t = cload(nc.sync, wr8[:, :, :], [P, 2, P], fp8, "wr8")
            swide = cload(nc.gpsimd, Sw_d[:, :], [P, 8 * WIDE], fp8, "swide")
            wm01_t = cload(nc.gpsimd, wm01[:, :, :], [P, 2, P], fp8, "wm01")
            dstT0 = cp.tile([P, 1024], bf16, tag="dstT0", name="dstT0")
            dstT1 = cp.tile([P, 1024], bf16, tag="dstT1", name="dstT1")
            for r in (0, 64):
                nc.sync.dma_start(out=dstT0[r:r + 64, :], in_=dstT[0, r:r + 64, :])
                nc.sync.dma_start(out=dstT1[r:r + 64, :], in_=dstT[1, r:r + 64, :])

            def dst_stage(b, agg_ps, stage, hold):
                dc = slice(b * P, (b + 1) * P)
                if stage == 0:
                    mmean = dmid.tile([P, P], bf16, tag="mmean")
                    nc.vector.tensor_copy(mmean[:], agg_ps[:, :P])
                    drp = dstps.tile([P, P], f32, tag="dst")
                    nc.tensor.matmul(drp[:], lhsT=wr0[:], rhs=dstT0[:, dc],
                                     start=True, stop=False)
                    nc.tensor.matmul(drp[:], lhsT=wr1[:], rhs=dstT1[:, dc],
                                     start=False, stop=True)
                    dstr = dmid.tile([P, P], bf16, tag="dstr")
                    nc.scalar.activation(dstr[:], drp[:], AF.Relu, bias=br_t[:, :1])
                    hold.update(mmean=mmean, dstr=dstr)
                elif stage == 1:
                    agp = dstps.tile([P, P], f32, tag="dst")
                    nc.tensor.matmul(agp[:], lhsT=wa0[:], rhs=hold["dstr"][:],
                                     start=True, stop=False)
                    nc.tensor.matmul(agp[:], lhsT=wa1[:], rhs=hold["mmean"][:],
                                     start=False, stop=True)
                    aggT = dmid.tile([P, P], bf16, tag="aggT")
                    nc.scalar.activation(aggT[:], agp[:], AF.Relu, bias=ba_t[:, :1])
                    hold.update(aggT=aggT)
                elif stage == 2:
                    upp = dstps.tile([P, P], f32, tag="dst")
                    nc.tensor.matmul(upp[:], lhsT=wu0[:], rhs=hold["aggT"][:],
                                     start=True, stop=False)
                    nc.tensor.matmul(upp[:], lhsT=wu1[:], rhs=hold["dstr"][:],
                                     start=False, stop=True)
                    updT = dmid.tile([P, P], bf16, tag="updT")
                    nc.scalar.activation(updT[:], upp[:], AF.Relu, bias=bu_t[:, :1])
                    hold.update(updT=updT)
                else:
                    wrp = dstps.tile([P, P], f32, tag="dst")
                    nc.tensor.matmul(wrp[:], lhsT=ww_t[:], rhs=hold["updT"][:],
                                     start=True, stop=True)
                    wout = dmid.tile([P, P], f32, tag="wout")
                    nc.scalar.activation(wout[:], wrp[:], AF.Tanh, bias=bw_t[:, :1])
                    nc.sync.dma_start(out=out_d[:, dc], in_=wout[:])

            def issue_chunk(ci):
                # DRAM slabs are chunk-contiguous.  s01/ef spread across the
                # SWDGE path (gpsimd) and the two HWDGE queues for balance.
                # mi plane 0 is filled by the read-MLP relu on ACT; plane 1
                # is the (zero-padded) edge-feature k-tile, DMA'd here.
                if ci >= nch:
                    return None
                s_lo = ci * CH
                s_hi = min(nsup, s_lo + CH)
                w_cols = (s_hi - s_lo) * SUP
                s01c = iop.tile([P, 2, CH * SUP], fp8, tag="s01")
                mi = iop.tile([P, 2, CH * SUP], fp8, tag="mi")
                Sc = iop.tile([P, CH * 4 * W], fp8, tag="Sc")
                if ci in (1, 5, 7):
                    for r in range(0, P, 64):
                        nc.sync.dma_start(out=s01c[r:r + 64, :, :w_cols],
                                          in_=s01_d[ci, r:r + 64, :, :w_cols])
                elif ci < 2:
                    # ramp: split so early super-tiles can start sooner
                    qn = CH * SUP // 4
                    for j in range(4):
                        c0, c1 = j * qn, min((j + 1) * qn, w_cols)
                        if c0 < c1:
                            nc.gpsimd.dma_start(out=s01c[:, :, c0:c1],
                                                in_=s01_d[ci, :, :, c0:c1])
                else:
                    nc.gpsimd.dma_start(out=s01c[:, :, :w_cols],
                                        in_=s01_d[ci, :, :, :w_cols])
                if ci % 2 == 0:
                    for (r0, r1) in ((0, 64), (64, P)):
                        nc.scalar.dma_start(out=mi[r0:r1, 1, :w_cols],
                                            in_=efts[ci, r0:r1, :w_cols])
                else:
                    nc.gpsimd.dma_start(out=mi[:, 1, :w_cols],
                                        in_=efts[ci, :, :w_cols])
                sw = (s_hi - s_lo) * 4 * W
                nc.sync.dma_start(out=Sc[:, :sw], in_=S_d[ci, :, :sw])
                return (s_lo, s_hi, s01c, mi, Sc)

            PF = 4                  # chunks issued ahead of compute
            chunks = [issue_chunk(0)]
            wr0 = cload(nc.gpsimd, wr[0, :, :], [P, P], bf16, "wr0")
            wr1 = cload(nc.gpsimd, wr[1, :, :], [P, P], bf16, "wr1")
            wa0 = cload(nc.gpsimd, wa[0, :, :], [P, P], bf16, "wa0")
            wa1 = cload(nc.gpsimd, wa[1, :, :], [P, P], bf16, "wa1")
            wu0 = cload(nc.gpsimd, wu[0, :, :], [P, P], bf16, "wu0")
            wu1 = cload(nc.gpsimd, wu[1, :, :], [P, P], bf16, "wu1")
            ww_t = cload(nc.gpsimd, ww[:, :], [P, P], bf16, "ww")
            ba_t = cload(nc.gpsimd, ba[:, :], [P, 1], f32, "ba")
            bu_t = cload(nc.gpsimd, bu[:, :], [P, 1], f32, "bu")
            bw_t = cload(nc.gpsimd, bw[:, :], [P, 1], f32, "bw")
            chunks += [issue_chunk(ci) for ci in range(1, PF)]

            pending = None          # [slot, agg_tile, next_stage]
            hold = {}
            cur_agg = None
            stage_q = []            # software pipeline: read -> msg -> agg
            st_ctx = {}

            def emit_msg(ent):
                s, nq, mi, Sc, s_lo, _ = ent
                sl_sup = (s - s_lo) * SUP
                mg = mgps.tile([P, SUP], f32, tag="mg")
                for q in range(nq):
                    qs = slice(q * P, (q + 1) * P)
                    nc.tensor.matmul(mg[:, qs],
                                     lhsT=mi[:, :, sl_sup + q * P:
                                             sl_sup + (q + 1) * P],
                                     rhs=wm01_t[:], start=True, stop=True,
                                     skip_group_check=True,
                                     perf_mode=mybir.MatmulPerfMode.DoubleRow)
                msgs = msp.tile([P, SUP], fp8, tag="msgs")
                nc.vector.tensor_scalar_max(msgs[:, :nq * P],
                                            mg[:, :nq * P], 0.0)
                st_ctx[s] = msgs

            def emit_agg(ent):
                nonlocal pending, hold, cur_agg
                s, nq, mi, Sc, s_lo, _ = ent
                msgs = st_ctx.pop(s)
                for q in range(nq):
                    t = s * 4 + q
                    k, qin, base = blk_of[t], qin_of[t], bases[t]
                    qs = slice(q * P, (q + 1) * P)
                    last = qin == caps[k] - 1
                    if qin == 0:
                        cur_agg = aggps.tile([P, WIDE], f32, tag="agg")
                        nc.tensor.matmul(cur_agg[:, :],
                                         lhsT=msgs[:, qs],
                                         rhs=swide[:, k * WIDE:(k + 1) * WIDE],
                                         start=True, stop=last,
                                         skip_group_check=True)
                    else:
                        nc.tensor.matmul(cur_agg[:, base:base + W],
                                         lhsT=msgs[:, qs],
                                         rhs=Sc[:, (t - s_lo * 4) * W:
                                                 (t - s_lo * 4 + 1) * W],
                                         start=False, stop=last,
                                         skip_group_check=True)
                    if pending is not None and qin in (2, 6, 10, 14):
                        dst_stage(pending[0], pending[1], pending[2], hold)
                        pending[2] += 1
                        if pending[2] == 4:
                            pending = None
                    if last:
                        if pending is not None:   # tiny-cap fallback: flush
                            while pending[2] < 4:
                                dst_stage(pending[0], pending[1],
                                          pending[2], hold)
                                pending[2] += 1
                        pending = [k, cur_agg, 0]
                        hold = {}

            for ci in range(nch):
                s_lo, s_hi, s01c, mi, Sc = chunks[ci]
                nxt = issue_chunk(ci + PF)
                if nxt is not None:
                    chunks.append(nxt)

                for s in range(s_lo, s_hi):
                    sl = s - s_lo
                    col = slice(sl * SUP, (sl + 1) * SUP)
                    nq = min(T - s * 4, 4)      # valid subtiles this super-tile
                    rd = rdps.tile([P, SUP], f32, tag="rd")
                    nc.tensor.matmul(rd[:], lhsT=wr8_t[:], rhs=s01c[:, :, col],
                                     start=True, stop=True,
                                     perf_mode=mybir.MatmulPerfMode.DoubleRow)
                    nc.scalar.activation(mi[:, 0, col], rd[:], AF.Relu,
                                         bias=br_t[:, :1])
                    ent = (s, nq, mi, Sc, s_lo, None)
                    emit_msg(ent)
                    emit_agg(ent)
            while pending[2] < 4:
                dst_stage(pending[0], pending[1], pending[2], hold)
                pending[2] += 1

    nc.finalize()
    return nc


def _pack(dest_seg, bounds, assign, caps, W):
    """Lockstep-pack each slot's 8 blocks into caps[k] subtiles with shared
    static window bases.  Returns (bases, takes) or None if W too narrow.
    takes[t] = list of (core, edge_lo, count, base)."""
    bases, takes = [], []
    for k in range(8):
        lo = np.array([bounds[assign[c, k]] for c in range(8)], np.int64)
        hi = np.array([bounds[assign[c, k] + 1] for c in range(8)], np.int64)
        blk0 = np.array([assign[c, k] * P for c in range(8)], np.int64)
        ptr = lo.copy()
        for q in range(caps[k]):
            pend = ptr < hi
            if pend.any():
                base = int(min(dest_seg[ptr[c]] - blk0[c]
                               for c in range(8) if pend[c]))
            else:
                base = 0
            if q == 0 or W >= P:
                base, weff = 0, P
            else:
                # window must stay inside the [0, WIDE) PSUM region
                base = min(base, WIDE - W)
                weff = W
            rec = []
            for c in range(8):
                if not pend[c]:
                    rec.append((c, int(ptr[c]), 0, base))
                    continue
                dl = dest_seg[ptr[c]:hi[c]] - blk0[c]
                nfit = int(np.searchsorted(dl, base + weff))
                tc = min(P, nfit)
                rec.append((c, int(ptr[c]), tc, base))
                ptr[c] += tc
            bases.append(base)
            takes.append(rec)
        if (ptr != hi).any():
            return None
    return bases, takes


def _prep_inputs(inputs):
    """Host-side shard/pack. Returns (in_maps, key, node_memory, node_ids,
    dest_cols)."""
    node_memory = np.ascontiguousarray(np.asarray(inputs["node_memory"], np.float32))
    node_features = np.asarray(inputs["node_features"], np.float32)
    edge_features = np.asarray(inputs["edge_features"], np.float32)
    time_encoding = np.asarray(inputs["time_encoding"], np.float32)
    node_ids = np.asarray(inputs["node_ids"]).astype(np.int64)
    source_ids = np.asarray(inputs["source_ids"]).astype(np.int64)
    edge_ids = np.asarray(inputs["edge_ids"]).astype(np.int64)
    dest_seg = np.asarray(inputs["dest_seg"]).astype(np.int64)
    W_read = np.asarray(inputs["W_read"], np.float32)
    b_read = np.asarray(inputs["b_read"], np.float32)
    W_msg = np.asarray(inputs["W_msg"], np.float32)
    b_msg = np.asarray(inputs["b_msg"], np.float32)
    W_agg = np.asarray(inputs["W_agg"], np.float32)
    b_agg = np.asarray(inputs["b_agg"], np.float32)
    W_upd = np.asarray(inputs["W_upd"], np.float32)
    b_upd = np.asarray(inputs["b_upd"], np.float32)
    W_write = np.asarray(inputs["W_write"], np.float32)
    b_write = np.asarray(inputs["b_write"], np.float32)

    n_edge = dest_seg.shape[0]

    cnt = np.bincount(dest_seg, minlength=N_DEST)
    inv_cnt = np.zeros(N_DEST, np.float32)
    nz = cnt > 0
    inv_cnt[nz] = 1.0 / cnt[nz]

    # 64 global dest blocks of 128 dests; sort desc by edge count into slots
    bounds = np.searchsorted(dest_seg, np.arange(0, N_DEST + 1, P))
    n_b = np.diff(bounds)
    order = np.argsort(-n_b, kind="stable")
    assign = np.zeros((8, 8), np.int64)     # [core, slot] -> global block
    for k in range(8):
        grp = np.sort(order[8 * k:8 * (k + 1)])
        assign[:, k] = grp
    caps = tuple(int(math.ceil(max(n_b[assign[c, k]] for c in range(8)) / P))
                 for k in range(8))

    packed = None
    for W in (32, 64, 128):
        packed = _pack(dest_seg, bounds, assign, caps, W)
        if packed is not None:
            break
    assert packed is not None
    bases, takes = packed
    T = len(bases)
    nsup = (T + 3) // 4
    T4 = nsup * 4
    e_cap = nsup * SUP

    blk_of, qin_of = [], []
    for k, ck in enumerate(caps):
        blk_of += [k] * ck
        qin_of += list(range(ck))

    # per-core flat edge selection
    esel = np.zeros((N_CORES, e_cap), np.int64)
    valid = np.zeros((N_CORES, e_cap), bool)
    for t, rec in enumerate(takes):
        for (c, elo, tc, base) in rec:
            if tc:
                esel[c, t * P:t * P + tc] = np.arange(elo, elo + tc)
                valid[c, t * P:t * P + tc] = True
    esel_f = esel.reshape(-1)
    valid_f = valid.reshape(-1)

    nodecat = np.concatenate([node_memory, node_features], axis=1)  # [N,256]

    nch = (nsup + CH - 1) // CH
    e_pad = nch * CH * SUP          # chunk-major padded edge capacity

    def chunk_major(arr):
        """[N_CORES, R, e_cap] -> [N_CORES, nch, R, CH*SUP] contiguous."""
        n, r = arr.shape[0], arr.shape[1]
        out = np.zeros((n, r, e_pad), arr.dtype)
        out[:, :, :e_cap] = arr
        return np.ascontiguousarray(
            out.reshape(n, r, nch, CH * SUP).transpose(0, 2, 1, 3))

    src_rows = nodecat[source_ids[esel_f]]
    src_rows[~valid_f] = 0.0
    srcT = np.ascontiguousarray(
        src_rows.reshape(N_CORES, e_cap, 256).transpose(0, 2, 1)
    ).astype(FP8).reshape(N_CORES, 2, P, e_cap)
    # planar k-pair layout for DoubleRow: [N, nch, P, 2, CH*SUP]
    s01_pad = np.zeros((N_CORES, 2, P, e_pad), FP8)
    s01_pad[:, :, :, :e_cap] = srcT
    s01 = np.ascontiguousarray(
        s01_pad.reshape(N_CORES, 2, P, nch, CH * SUP)
        .transpose(0, 3, 2, 1, 4))

    ef_rows = edge_features[edge_ids[esel_f]]
    t_rows = time_encoding[np.minimum(esel_f, n_edge - 1)]
    eft = np.concatenate(
        [ef_rows, t_rows, np.ones((len(esel_f), 1), np.float32),
         np.zeros((len(esel_f), 31), np.float32)], axis=1)
    eft[~valid_f] = 0.0
    eft[valid_f, 96] = 1.0
    efts = np.ascontiguousarray(
        eft.reshape(N_CORES, e_cap, P).transpose(0, 2, 1)).astype(FP8)
    efts = chunk_major(efts)                      # [N, nch, 128, CH*SUP]

    # windowed scaled one-hot S + per-block wide first-subtile slice
    W_used = W          # width that succeeded in the pack loop above
    S_np = np.zeros((N_CORES, P, nch * CH * 4 * W_used), np.float32)
    Sw_np = np.zeros((N_CORES, P, 8 * WIDE), np.float32)
    for t, rec in enumerate(takes):
        k, qin = blk_of[t], qin_of[t]
        for (c, elo, tc, base) in rec:
            if not tc:
                continue
            dl = (dest_seg[elo:elo + tc] - assign[c, k] * P).astype(np.int64)
            sc = inv_cnt[dest_seg[elo:elo + tc]]
            e_i = np.arange(tc)
            if qin == 0:
                Sw_np[c, e_i, k * WIDE + dl] = sc
            else:
                S_np[c, e_i, t * W_used + (dl - base)] = sc
    S_np = np.ascontiguousarray(
        S_np.reshape(N_CORES, P, nch, CH * 4 * W_used).transpose(0, 2, 1, 3)
    ).astype(FP8)                                 # [N, nch, P, CH*4*W]
    Sw_np = Sw_np.astype(FP8)

    # dst-side node rows, in slot order per core
    dest_cols = np.zeros((N_CORES, 1024), np.int64)   # dest index per out col
    for c in range(N_CORES):
        for k in range(8):
            dest_cols[c, k * P:(k + 1) * P] = assign[c, k] * P + np.arange(P)
    drows = nodecat[node_ids[dest_cols.reshape(-1)]]
    dstT = np.ascontiguousarray(
        drows.reshape(N_CORES, 1024, 256).transpose(0, 2, 1)
    ).astype(BF16).reshape(N_CORES, 2, P, 1024)

    wr_h = np.ascontiguousarray(W_read.reshape(2, P, P)).astype(BF16)
    wr8_h = np.ascontiguousarray(
        W_read.reshape(2, P, P).transpose(1, 0, 2)).astype(FP8)  # [P,2,P]
    wm01_h = np.zeros((P, 2, P), np.float32)
    wm01_h[:, 0, :] = W_msg[:P]
    wm01_h[:97, 1, :] = np.concatenate([W_msg[P:], b_msg[None, :]], axis=0)
    wm01_h = wm01_h.astype(FP8)
    wa_h = np.ascontiguousarray(W_agg.reshape(2, P, P)).astype(BF16)
    wu_h = np.ascontiguousarray(W_upd.reshape(2, P, P)).astype(BF16)
    ww_h = np.ascontiguousarray(W_write).astype(BF16)
    br_h = np.ascontiguousarray(b_read[:, None]).astype(np.float32)
    ba_h = np.ascontiguousarray(b_agg[:, None]).astype(np.float32)
    bu_h = np.ascontiguousarray(b_upd[:, None]).astype(np.float32)
    bw_h = np.ascontiguousarray(b_write[:, None]).astype(np.float32)

    in_maps = []
    for c in range(N_CORES):
        in_maps.append({
            "s01": s01[c], "efts": efts[c], "S_d": S_np[c], "Sw_d": Sw_np[c],
            "dstT": dstT[c],
            "wr": wr_h, "wr8": wr8_h, "wm01": wm01_h, "wa": wa_h, "wu": wu_h,
            "ww": ww_h, "br": br_h, "ba": ba_h, "bu": bu_h, "bw": bw_h,
        })
    key = (caps, tuple(bases), W_used)
    return in_maps, key, node_memory, node_ids, dest_cols


def run(inputs, trace=False, **kw):
    in_maps, key, node_memory, node_ids, dest_cols = _prep_inputs(inputs)
    if key not in _PROG_CACHE:
        _PROG_CACHE[key] = _build_program(key[0], key[1], key[2])
    nc = _PROG_CACHE[key]
    res = run_bass_kernel_spmd(nc, in_maps, core_ids=list(range(N_CORES)),
                               trace=trace, **kw)
    out = node_memory.copy()
    for c in range(N_CORES):
        wt = np.asarray(res.results[c]["writeT"], np.float32).T  # [1024,128]
        out[node_ids[dest_cols[c]]] = wt
    return out, res


def kernel(**inputs) -> np.ndarray:
    out, _ = run(inputs, trace=False)
    return out
